# revision 54
# baseline (speedup 1.0000x reference)
"""KgAdapterCrossAttention kernel for 8 trn2 NeuronCores.

Sharding: core = (batch b, query-half qh).  Each core computes attention for
1024 queries of one batch element against all 2048 keys.  221us -> 81us vs
the fp32 baseline (2.7x), all numerics within 1e-2 of the fp32 reference.

Design notes:
  - All matmuls use float32r / bf16 operands: 1 cycle/row on the PE instead
    of fp32's 4 (f32r keeps fp32 accuracy; inputs arrive as f32r via DMA,
    PSUM->SBUF copies act as the required f32r rounding ops).
  - Scores are computed pre-scaled by log2e*128 (folded into Wq on the host).
  - The 16 k-tiles of each (query-block, head) are processed in PAIRS that
    share a [128,1024] PSUM tile (two banks, sequential accumulation groups)
    so each exp instruction covers 1024 columns, amortizing the ~150ns
    access-latency overhead of Act/DVE instructions.  3 pair-slots rotate so
    both exp engines stream without stalling on PSUM.
  - exp splits across engines (pair schedule D,A,A,D,A,A,D,A): Act pairs use
    native Exp (scale=1/(log2e*128)) followed by a gpsimd align-mask
    multiply; DVE pairs use a fused fast-exp: ONE tensor_add of the PSUM
    scores with amq = mask ? 16250 : 8192 (i16), truncated to i16, whose
    bf16 BITCAST equals exp2(s*log2e) with the mask folded in (masked lanes
    land at ~2^-63).  16250 rather than 16256 centers the exponent-trick's
    linear-interpolation error (+-3%, zero mean, cancels in softmax).
  - P*V chains run per (qt, head); qt0 interleaves one head behind the score
    pipeline, qt1..3 defer into the NEXT query-block's head phases (pt tiles
    for heads 0/1 are double-buffered across blocks to allow the overlap).
    The softmax denominator rides along as a ones-column in V; normalize
    folds into the PSUM->SBUF copy as a per-partition scalar multiply.
  - O-projection transposes write into spare space of the O PSUM bank; the
    tail's qt2/3 accumulators borrow idle score-pool slots.
  - Mask DMAs are chunked per consumption unit and interleaved with the
    activation loads so arrivals pace the pipeline start.
"""

import os
import sys

import numpy as np
import ml_dtypes

try:
    import concourse.bass as bass
except ImportError:
    for _p in ("/opt/trn_rl_repo", os.path.expanduser("~/.axon_site/_ro/trn_rl_repo")):
        if os.path.isdir(_p) and _p not in sys.path:
            sys.path.insert(0, _p)
    import concourse.bass as bass

import concourse.mybir as mybir
import concourse.tile as tile
from concourse import bacc
from concourse.masks import make_identity
from contextlib import ExitStack

F32 = mybir.dt.float32
F32R = mybir.dt.float32r
BF16 = mybir.dt.bfloat16
I16 = mybir.dt.int16
EXP = mybir.ActivationFunctionType.Exp
ALU = mybir.AluOpType

P = 128
HID = 256
NHEAD = 4
DHEAD = 64
NQ = 1024  # queries per core
NK = 2048  # keys (full)
QBLK = 512
NQB = NQ // QBLK  # 2
NKT = NK // P  # 16
NPAIR = NKT // 2  # 8
NCT = HID // P  # 2

FE_MUL = float(np.log2(np.e)) * 128.0  # folded into Wq on host
ACT_SCALE = 1.0 / FE_MUL
AMQ_KEEP = 16250  # 127*128 minus centering delta 6
AMQ_KILL = 8192   # masked lanes -> bf16 2^-63 ~ 0

# engine per kt-pair and per query-block: D = DVE fast-exp, A = Act exp.
# qb0 runs 4/4 (DVE has no deferred-tail work there); qb1 runs 5A/3D since
# qb0's deferred tails land on DVE during qb1's phases.
PAIR_ENG_QB = [
    list(os.environ.get("KG_PAIR_ENG0", "ADADADAD")),
    list(os.environ.get("KG_PAIR_ENG1", "ADAADADA")),
]


def build() -> bass.Bass:
    nc = bacc.Bacc()
    xqT = nc.declare_dram_parameter("xqT", [HID, NQ], BF16, isOutput=False)
    xkT = nc.declare_dram_parameter("xkT", [HID, NK], BF16, isOutput=False)
    amf = nc.declare_dram_parameter("amf", [NKT * P, NQ], BF16, isOutput=False)
    amq = nc.declare_dram_parameter("amq", [NKT * P, NQ], I16, isOutput=False)
    wqT = nc.declare_dram_parameter("wqT", [HID, HID], BF16, isOutput=False)
    wkT = nc.declare_dram_parameter("wkT", [HID, HID], BF16, isOutput=False)
    wvT = nc.declare_dram_parameter("wvT", [HID, HID], BF16, isOutput=False)
    woT = nc.declare_dram_parameter("woT", [HID, HID], BF16, isOutput=False)
    out_d = nc.declare_dram_parameter("out", [NQ, HID], F32, isOutput=True)

    with tile.TileContext(nc) as tc, ExitStack() as ctx:
        const = ctx.enter_context(tc.tile_pool(name="const", bufs=1))
        big = ctx.enter_context(tc.tile_pool(name="big", bufs=1))
        ptp = ctx.enter_context(tc.tile_pool(name="ptp", bufs=1))
        amp = ctx.enter_context(tc.tile_pool(name="amp", bufs=1))
        wrk = ctx.enter_context(tc.tile_pool(name="wrk", bufs=2))
        wrk1 = ctx.enter_context(tc.tile_pool(name="wrk1", bufs=1))
        ps_st = ctx.enter_context(tc.tile_pool(name="ps_st", bufs=3, space="PSUM"))
        ps_a = ctx.enter_context(tc.tile_pool(name="ps_a", bufs=1, space="PSUM"))
        ps_o = ctx.enter_context(tc.tile_pool(name="ps_o", bufs=1, space="PSUM"))

        # --- DMA loads (two hwdge queues: SP carries the K side, Act the Q
        # side, so K-proj and Q-proj inputs stream in parallel) ---
        def load2(name, src, width, dt=F32R, dma_eng=None):
            # one DMA for both 128-row tiles: small weight loads are
            # descriptor-bound (max(500ns, transfer)), so merging halves cost
            tl = const.tile([P, 2, width], dt, tag=name, name=name)
            (dma_eng or nc.sync).dma_start(
                out=tl, in_=src.rearrange("(t p) c -> p t c", p=P)
            )
            return [tl[:, t, :] for t in range(2)]

        wk_sb = load2("wk", wkT, HID, dt=BF16)
        warm_in = const.tile([P, 1], F32, tag="warm_in", name="warm_in")
        nc.gpsimd.memset(warm_in, 0.0)
        warm_out = const.tile([P, 1], BF16, tag="warm_out", name="warm_out")
        nc.scalar.activation(warm_out, warm_in, EXP)
        # x tiles hold both 128-row halves ([P, 2, N]) so one DMA chunk
        # carries exactly one matmul's worth of contraction input
        xk2 = big.tile([P, 2, NK], BF16, tag="xk", name="xk")
        xq2 = big.tile([P, 2, NQ], BF16, tag="xq", name="xq")
        xk_sb = [xk2[:, t, :] for t in range(2)]
        xq_sb = [xq2[:, t, :] for t in range(2)]
        xkT_r = xkT.rearrange("(t p) c -> p t c", p=P)
        xqT_r = xqT.rearrange("(t p) c -> p t c", p=P)

        wq_sb = load2("wq", wqT, HID, dt=BF16, dma_eng=nc.scalar)
        for c in range(2):
            nc.scalar.dma_start(
                out=xq2[:, :, c * QBLK : (c + 1) * QBLK],
                in_=xqT_r[:, :, c * QBLK : (c + 1) * QBLK],
            )
        for c in range(NK // QBLK):
            nc.sync.dma_start(
                out=xk2[:, :, c * QBLK : (c + 1) * QBLK],
                in_=xkT_r[:, :, c * QBLK : (c + 1) * QBLK],
            )
        wv_sb = load2("wv", wvT, HID, dt=BF16)

        amf_r = amf.rearrange("(t p) q -> p t q", p=P)
        amq_r = amq.rearrange("(t p) q -> p t q", p=P)
        am_sb = {}   # (qb, pj) -> tile [P, 2, QBLK] bf16
        amq_sb = {}  # (qb, pj) -> tile [P, 2, QBLK] i16

        def load_mask_chunks(qb, dma_eng=None):
            # one chunk per consumption unit, in pair order, so the DMA
            # arrivals pace the pipeline.  qb0's chunks ride the gpsimd
            # queue (idle during the prologue) so the SP queue stays short.
            # qb1's chunks prefetch into their own pool while SP is idle.
            dma_eng = dma_eng or nc.sync
            pool = amp
            qsl = slice(qb * QBLK, (qb + 1) * QBLK)
            for pj in range(NPAIR):
                eng = PAIR_ENG_QB[qb][pj]
                if eng == "D":
                    tq = pool.tile(
                        [P, 2, QBLK], I16, tag=f"amq_{pj}", name=f"amq_{pj}"
                    )
                    dma_eng.dma_start(out=tq, in_=amq_r[:, 2 * pj : 2 * pj + 2, qsl])
                    amq_sb[(qb, pj)] = tq
                else:
                    tl = pool.tile(
                        [P, 2, QBLK], BF16, tag=f"am_{pj}", name=f"am_{pj}"
                    )
                    dma_eng.dma_start(out=tl, in_=amf_r[:, 2 * pj : 2 * pj + 2, qsl])
                    am_sb[(qb, pj)] = tl

        load_mask_chunks(0, dma_eng=nc.gpsimd)
        wo_sb = load2("wo", woT, HID, dt=BF16)

        ident_f = const.tile([P, P], F32, tag="ident_f", name="ident_f")
        make_identity(nc, ident_f)
        ident = const.tile([P, P], BF16, tag="ident", name="ident")
        nc.vector.tensor_copy(ident, ident_f)

        # --- projections (copies alternate Act/DVE; prologue work) ---
        cp_i = 0

        def copy_eng(out, in_):
            nonlocal cp_i
            cp_i += 1
            if cp_i % 2 == 0:
                nc.scalar.copy(out, in_)
            else:
                nc.vector.tensor_copy(out, in_)

        kt_sb = [big.tile([P, NK], F32R, tag=f"kt{t}", name=f"kt{t}") for t in range(2)]
        for nb in range(NK // (2 * QBLK)):
            for t in range(2):
                ps = ps_st.tile([P, 2 * QBLK], F32, tag="st", name="st")
                for half in range(2):
                    for ct in range(NCT):
                        nc.tensor.matmul(
                            ps[:, half * QBLK : (half + 1) * QBLK],
                            lhsT=wk_sb[ct][:, t * P : (t + 1) * P],
                            rhs=xk_sb[ct][
                                :, (2 * nb + half) * QBLK : (2 * nb + half + 1) * QBLK
                            ],
                            start=(ct == 0),
                            stop=(ct == NCT - 1),
                        )
                copy_eng(kt_sb[t][:, 2 * nb * QBLK : (2 * nb + 2) * QBLK], ps)

        qt_sb = [big.tile([P, NQ], F32R, tag=f"qt{t}", name=f"qt{t}") for t in range(2)]
        for t in range(2):
            ps = ps_st.tile([P, 2 * QBLK], F32, tag="st", name="st")
            for half in range(2):
                for ct in range(NCT):
                    nc.tensor.matmul(
                        ps[:, half * QBLK : (half + 1) * QBLK],
                        lhsT=wq_sb[ct][:, t * P : (t + 1) * P],
                        rhs=xq_sb[ct][:, half * QBLK : (half + 1) * QBLK],
                        start=(ct == 0),
                        stop=(ct == NCT - 1),
                    )
            copy_eng(qt_sb[t], ps)

        # V''[pair][ktok, half, h, 0:64] = V rows (bf16); [..., 64] = 1.0
        vpp2 = [None] * (NKT // 2)
        for kt2 in range(NKT // 2):
            ps = ps_st.tile([P, 2 * QBLK], F32, tag="st", name="st")
            for half in range(2):
                kt = 2 * kt2 + half
                for ct in range(NCT):
                    nc.tensor.matmul(
                        ps[:, half * QBLK : half * QBLK + HID],
                        lhsT=xk_sb[ct][:, kt * P : (kt + 1) * P],
                        rhs=wv_sb[ct],
                        start=(ct == 0),
                        stop=(ct == NCT - 1),
                    )
            tl = big.tile(
                [P, 2, NHEAD, DHEAD + 1], BF16, tag=f"v{kt2}", name=f"v{kt2}"
            )
            copy_eng(
                tl[:, :, :, 0:DHEAD],
                ps.rearrange("p (two q) -> p two q", two=2)[:, :, 0:HID].rearrange(
                    "p two (h d) -> p two h d", h=NHEAD
                ),
            )
            nc.gpsimd.memset(tl[:, :, :, DHEAD : DHEAD + 1], 1.0)
            vpp2[kt2] = tl

        # --- attention ---
        def emit_head(qb, h, pts):
            qsl = slice(qb * QBLK, (qb + 1) * QBLK)
            t, po = h // 2, (h % 2) * DHEAD
            for pj in range(NPAIR):
                ps = ps_st.tile([P, 2 * QBLK], F32, tag="st", name="st")
                for half in range(2):
                    kt = 2 * pj + half
                    nc.tensor.matmul(
                        ps[:, half * QBLK : (half + 1) * QBLK],
                        lhsT=kt_sb[t][po : po + DHEAD, kt * P : (kt + 1) * P],
                        rhs=qt_sb[t][po : po + DHEAD, qsl],
                        start=True,
                        stop=True,
                    )
                eng = PAIR_ENG_QB[qb][pj]
                tg = f"pt{h}{qb % 2}_{pj}"
                if eng == "A":
                    pt = ptp.tile([P, 2 * QBLK], BF16, tag=tg, name=tg)
                    nc.scalar.activation(pt, ps, EXP, scale=ACT_SCALE)
                    nc.gpsimd.tensor_mul(
                        pt.rearrange("p (two q) -> p two q", two=2),
                        pt.rearrange("p (two q) -> p two q", two=2),
                        am_sb[(qb, pj)],
                    )
                    for half in range(2):
                        pts[(h, 2 * pj + half)] = pt[:, half * QBLK : (half + 1) * QBLK]
                else:
                    ie = ptp.tile([P, 2 * QBLK], I16, tag=tg, name=tg)
                    nc.vector.tensor_add(
                        ie.rearrange("p (two q) -> p two q", two=2),
                        ps.rearrange("p (two q) -> p two q", two=2),
                        amq_sb[(qb, pj)],
                    )
                    pv = ie.bitcast(BF16)
                    for half in range(2):
                        pts[(h, 2 * pj + half)] = pv[:, half * QBLK : (half + 1) * QBLK]

        deferred = []

        def run_deferred():
            if deferred:
                deferred.pop(0)()

        def process_qb(qb):
            last = qb == NQB - 1
            if qb > 0:
                load_mask_chunks(qb)
            pts = {}
            ps_avs = {}
            ans = {}

            def get_ps_av(qt):
                if qt not in ps_avs:
                    # last qb: qt1 borrows a freed score slot so its chains
                    # don't serialize behind qt0's norm (shared "a0" tag)
                    if qt < 2 and not (last and qt == 1):
                        ps_avs[qt] = ps_a.tile(
                            [P, NHEAD * (DHEAD + 1)], F32, tag="a0",
                            name="a0", padded_shape=[P, QBLK],
                        )
                    else:
                        # tail-only: borrow an idle score-pool slot
                        big_t = ps_st.tile([P, 2 * QBLK], F32, tag="st", name="st_a")
                        ps_avs[qt] = big_t[:, 0 : NHEAD * (DHEAD + 1)]
                return ps_avs[qt]

            def emit_chain(qt, h):
                ps_av = get_ps_av(qt)
                for kt in range(NKT):
                    nc.tensor.matmul(
                        ps_av[:, h * 65 : (h + 1) * 65],
                        lhsT=pts[(h, kt)][:, qt * P : (qt + 1) * P],
                        rhs=vpp2[kt // 2][:, kt % 2, h, :],
                        start=(kt == 0),
                        stop=(kt == NKT - 1),
                    )

            def emit_norm(qt, on_act=False):
                # on_act: route the per-head normalize multiplies to the Act
                # engine (idle during the last qb's tail) so they overlap
                # DVE's att/ob copies
                ps_av = ps_avs.pop(qt)
                rec = wrk.tile([P, NHEAD], F32, tag=f"rec{qt % 2}", name=f"rec{qt % 2}")
                nc.vector.reciprocal(rec, ps_av[:, DHEAD : NHEAD * 65 : 65])
                an = wrk1.tile([P, HID], BF16, tag=f"an{qt % 2}", name=f"an{qt % 2}")
                for h in range(NHEAD):
                    if on_act:
                        nc.scalar.activation(
                            an[:, h * DHEAD : (h + 1) * DHEAD],
                            ps_av[:, h * 65 : h * 65 + DHEAD],
                            mybir.ActivationFunctionType.Copy,
                            scale=rec[:, h : h + 1],
                        )
                    else:
                        nc.vector.tensor_scalar_mul(
                            an[:, h * DHEAD : (h + 1) * DHEAD],
                            ps_av[:, h * 65 : h * 65 + DHEAD],
                            rec[:, h : h + 1],
                        )
                return an

            def emit_o(qt, an, on_act=False, borrow_ps=False, ob_act=False):
                # borrow_ps: use an idle score-pool slot instead of ps_o so two
                # O-chains can run concurrently (tail only, scores finished).
                # on_act/ob_act: route the att/ob copies to Act to parallelize
                # with DVE's stream.
                if borrow_ps:
                    big_t = ps_st.tile([P, 2 * QBLK], F32, tag="st", name="st_o")
                    o_ps = big_t[:, 0:QBLK]
                else:
                    o_ps = ps_o.tile([P, QBLK], F32, tag="o", name="o_ps")
                for ct in range(NCT):
                    tp = o_ps[:, HID + ct * DHEAD : HID + (ct + 1) * DHEAD].bitcast(BF16)
                    nc.tensor.transpose(tp, an[:, ct * P : (ct + 1) * P], ident)
                att = wrk.tile([P, HID], BF16, tag="att", name="att")
                if on_act:
                    nc.scalar.copy(att, o_ps[:, HID : HID + P].bitcast(BF16))
                else:
                    nc.vector.tensor_copy(att, o_ps[:, HID : HID + P].bitcast(BF16))
                for ct in range(NCT):
                    nc.tensor.matmul(
                        o_ps[:, 0:HID],
                        lhsT=att[:, ct * P : (ct + 1) * P],
                        rhs=wo_sb[ct],
                        start=(ct == 0),
                        stop=(ct == NCT - 1),
                    )
                ob = wrk1.tile([P, HID], F32, tag=f"ob{qt % 2}", name=f"ob{qt % 2}")
                q0 = qb * QBLK + qt * P
                if ob_act == "split":
                    # final output: copy + DMA in halves on both engines /
                    # both hwdge queues so the end-of-kernel DMA drain overlaps
                    nc.scalar.copy(ob[:, 0 : HID // 2], o_ps[:, 0 : HID // 2])
                    nc.scalar.dma_start(
                        out=out_d[q0 : q0 + P, 0 : HID // 2], in_=ob[:, 0 : HID // 2]
                    )
                    nc.vector.tensor_copy(
                        ob[:, HID // 2 : HID], o_ps[:, HID // 2 : HID]
                    )
                    nc.sync.dma_start(
                        out=out_d[q0 : q0 + P, HID // 2 : HID],
                        in_=ob[:, HID // 2 : HID],
                    )
                    return
                if ob_act:
                    nc.scalar.copy(ob, o_ps[:, 0:HID])
                else:
                    nc.vector.tensor_copy(ob, o_ps[:, 0:HID])
                dma_eng = nc.scalar if ob_act else nc.sync
                dma_eng.dma_start(out=out_d[q0 : q0 + P, :], in_=ob)

            for h in range(NHEAD):
                emit_head(qb, h, pts)
                if h in (1, 2, 3):
                    # previous qb's deferred tail chains/outputs overlap
                    # this qb's head phases (three slices to spread the
                    # extra PE load; pt tiles are double-buffered across qbs)
                    run_deferred()
                if h > 0:
                    emit_chain(0, h - 1)
            emit_chain(0, NHEAD - 1)
            an0 = emit_norm(0)

            def tail1(emit_chain=emit_chain, emit_norm=emit_norm,
                      emit_o=emit_o, an0=an0, last=last):
                emit_o(0, an0, on_act=last, ob_act=last)
                if last:
                    return  # the rest is emitted in tail2's interleaved form
                for h in range(NHEAD):
                    emit_chain(1, h)

            def tail1b(emit_chain=emit_chain, emit_norm=emit_norm,
                       emit_o=emit_o):
                an1 = emit_norm(1)
                emit_o(1, an1)
                for h in range(NHEAD):
                    emit_chain(2, h)

            def tail2(emit_chain=emit_chain, emit_norm=emit_norm,
                      emit_o=emit_o, last=last):
                if not last:
                    an2 = emit_norm(2)
                    for h in range(NHEAD):
                        emit_chain(3, h)
                    emit_o(2, an2)
                    an3 = emit_norm(3)
                    emit_o(3, an3)
                    return
                # Last qb: one interleaved sequence.  Engines are in-order, so
                # emission order per engine is chosen to avoid ready-op
                # head-of-line blocking: DVE gets recips early; norms/copies
                # alternate Act and DVE per qt; PE O-work slots between chain
                # groups.
                for h in range(NHEAD):
                    emit_chain(1, h)
                an1 = emit_norm(1, on_act=True)      # recip1 DVE, norms Act
                for h in range(NHEAD):
                    emit_chain(2, h)
                emit_o(1, an1, on_act=True, ob_act=True)
                an2 = emit_norm(2)                    # recip2+norms DVE
                for h in range(NHEAD):
                    emit_chain(3, h)
                emit_o(2, an2, on_act=True, ob_act=True)  # Act lane
                an3 = emit_norm(3)                    # recip3+norms DVE
                emit_o(3, an3, borrow_ps=True)        # DVE lane, DMA SP

            deferred.append(tail1)
            if not last:
                deferred.append(tail1b)
            deferred.append(tail2)

        for qb in range(NQB):
            process_qb(qb)
        run_deferred()
        run_deferred()
    nc.compile()
    return nc


_NC_CACHE = {}
_last_in_maps = None


def _get_nc(with_attn_mask: bool = False) -> bass.Bass:
    key = "v5"
    if key not in _NC_CACHE:
        _NC_CACHE[key] = build()
    return _NC_CACHE[key]


def kernel(q_hidden_states, k_hidden_states, attention_mask, align_mask, Wq, Wk, Wv, Wo):
    from concourse.bass_utils import run_bass_kernel_spmd

    q_hidden_states = np.asarray(q_hidden_states, np.float32)
    k_hidden_states = np.asarray(k_hidden_states, np.float32)
    attention_mask = np.asarray(attention_mask, np.float32)
    align_mask = np.asarray(align_mask)
    B, Q, _ = q_hidden_states.shape
    qh_len = Q // 2  # 1024

    nc = _get_nc()

    # scores arrive in PSUM pre-scaled by log2e*128 (folded into Wq here)
    wq = np.ascontiguousarray(
        (np.asarray(Wq, np.float32).T * np.float32(FE_MUL / 8.0)).astype(ml_dtypes.bfloat16)
    )
    wk = np.ascontiguousarray(np.asarray(Wk, np.float32).T.astype(ml_dtypes.bfloat16))
    wv = np.ascontiguousarray(np.asarray(Wv, np.float32).T.astype(ml_dtypes.bfloat16))
    wo = np.ascontiguousarray(np.asarray(Wo, np.float32).T.astype(ml_dtypes.bfloat16))

    if np.any(attention_mask):
        raise NotImplementedError("nonzero additive attention_mask not supported")

    in_maps = []
    for core in range(8):
        b, qh = divmod(core, 2)
        qsl = slice(qh * qh_len, (qh + 1) * qh_len)
        am = align_mask[b, :, qsl]
        m = {
            "xqT": np.ascontiguousarray(q_hidden_states[b, qsl].T.astype(ml_dtypes.bfloat16)),
            "xkT": np.ascontiguousarray(k_hidden_states[b].T.astype(ml_dtypes.bfloat16)),
            "amf": np.ascontiguousarray(am.astype(ml_dtypes.bfloat16)),
            "amq": np.ascontiguousarray(
                np.where(am != 0, AMQ_KEEP, AMQ_KILL).astype(np.int16)
            ),
            "wqT": wq,
            "wkT": wk,
            "wvT": wv,
            "woT": wo,
        }
        in_maps.append(m)

    global _last_in_maps
    _last_in_maps = in_maps
    res = run_bass_kernel_spmd(nc, in_maps, list(range(8))).results
    out = np.empty((B, Q, HID), np.float32)
    for core in range(8):
        b, qh = divmod(core, 2)
        out[b, qh * qh_len : (qh + 1) * qh_len] = res[core]["out"]
    return out



# revision 55
# speedup vs baseline: 1.0078x; 1.0078x over previous
"""KgAdapterCrossAttention kernel for 8 trn2 NeuronCores.

Sharding: core = (batch b, query-half qh).  Each core computes attention for
1024 queries of one batch element against all 2048 keys.  221us -> 81us vs
the fp32 baseline (2.7x), all numerics within 1e-2 of the fp32 reference.

Design notes:
  - All matmuls use float32r / bf16 operands: 1 cycle/row on the PE instead
    of fp32's 4 (f32r keeps fp32 accuracy; inputs arrive as f32r via DMA,
    PSUM->SBUF copies act as the required f32r rounding ops).
  - Scores are computed pre-scaled by log2e*128 (folded into Wq on the host).
  - The 16 k-tiles of each (query-block, head) are processed in PAIRS that
    share a [128,1024] PSUM tile (two banks, sequential accumulation groups)
    so each exp instruction covers 1024 columns, amortizing the ~150ns
    access-latency overhead of Act/DVE instructions.  3 pair-slots rotate so
    both exp engines stream without stalling on PSUM.
  - exp splits across engines (pair schedule D,A,A,D,A,A,D,A): Act pairs use
    native Exp (scale=1/(log2e*128)) followed by a gpsimd align-mask
    multiply; DVE pairs use a fused fast-exp: ONE tensor_add of the PSUM
    scores with amq = mask ? 16250 : 8192 (i16), truncated to i16, whose
    bf16 BITCAST equals exp2(s*log2e) with the mask folded in (masked lanes
    land at ~2^-63).  16250 rather than 16256 centers the exponent-trick's
    linear-interpolation error (+-3%, zero mean, cancels in softmax).
  - P*V chains run per (qt, head); qt0 interleaves one head behind the score
    pipeline, qt1..3 defer into the NEXT query-block's head phases (pt tiles
    for heads 0/1 are double-buffered across blocks to allow the overlap).
    The softmax denominator rides along as a ones-column in V; normalize
    folds into the PSUM->SBUF copy as a per-partition scalar multiply.
  - O-projection transposes write into spare space of the O PSUM bank; the
    tail's qt2/3 accumulators borrow idle score-pool slots.
  - Mask DMAs are chunked per consumption unit and interleaved with the
    activation loads so arrivals pace the pipeline start.
"""

import os
import sys

import numpy as np
import ml_dtypes

try:
    import concourse.bass as bass
except ImportError:
    for _p in ("/opt/trn_rl_repo", os.path.expanduser("~/.axon_site/_ro/trn_rl_repo")):
        if os.path.isdir(_p) and _p not in sys.path:
            sys.path.insert(0, _p)
    import concourse.bass as bass

import concourse.mybir as mybir
import concourse.tile as tile
from concourse import bacc
from concourse.masks import make_identity
from contextlib import ExitStack

F32 = mybir.dt.float32
F32R = mybir.dt.float32r
BF16 = mybir.dt.bfloat16
I16 = mybir.dt.int16
EXP = mybir.ActivationFunctionType.Exp
ALU = mybir.AluOpType

P = 128
HID = 256
NHEAD = 4
DHEAD = 64
NQ = 1024  # queries per core
NK = 2048  # keys (full)
QBLK = 512
NQB = NQ // QBLK  # 2
NKT = NK // P  # 16
NPAIR = NKT // 2  # 8
NCT = HID // P  # 2

FE_MUL = float(np.log2(np.e)) * 128.0  # folded into Wq on host
ACT_SCALE = 1.0 / FE_MUL
AMQ_KEEP = 16250  # 127*128 minus centering delta 6
AMQ_KILL = 8192   # masked lanes -> bf16 2^-63 ~ 0

# engine per kt-pair and per query-block: D = DVE fast-exp, A = Act exp.
# qb0 runs 4/4 (DVE has no deferred-tail work there); qb1 runs 5A/3D since
# qb0's deferred tails land on DVE during qb1's phases.
PAIR_ENG_QB = [
    list(os.environ.get("KG_PAIR_ENG0", "ADADADAD")),
    list(os.environ.get("KG_PAIR_ENG1", "ADAADADA")),
]


def build() -> bass.Bass:
    nc = bacc.Bacc()
    xqT = nc.declare_dram_parameter("xqT", [HID, NQ], BF16, isOutput=False)
    xkT = nc.declare_dram_parameter("xkT", [HID, NK], BF16, isOutput=False)
    amf = nc.declare_dram_parameter("amf", [NKT * P, NQ], BF16, isOutput=False)
    amq = nc.declare_dram_parameter("amq", [NKT * P, NQ], I16, isOutput=False)
    wqT = nc.declare_dram_parameter("wqT", [HID, HID], BF16, isOutput=False)
    wkT = nc.declare_dram_parameter("wkT", [HID, HID], BF16, isOutput=False)
    wvT = nc.declare_dram_parameter("wvT", [HID, HID], BF16, isOutput=False)
    woT = nc.declare_dram_parameter("woT", [HID, HID], BF16, isOutput=False)
    out_d = nc.declare_dram_parameter("out", [NQ, HID], F32, isOutput=True)

    with tile.TileContext(nc) as tc, ExitStack() as ctx:
        const = ctx.enter_context(tc.tile_pool(name="const", bufs=1))
        big = ctx.enter_context(tc.tile_pool(name="big", bufs=1))
        ptp = ctx.enter_context(tc.tile_pool(name="ptp", bufs=1))
        amp = ctx.enter_context(tc.tile_pool(name="amp", bufs=1))
        wrk = ctx.enter_context(tc.tile_pool(name="wrk", bufs=2))
        wrk1 = ctx.enter_context(tc.tile_pool(name="wrk1", bufs=1))
        ps_st = ctx.enter_context(tc.tile_pool(name="ps_st", bufs=3, space="PSUM"))
        ps_a = ctx.enter_context(tc.tile_pool(name="ps_a", bufs=1, space="PSUM"))
        ps_o = ctx.enter_context(tc.tile_pool(name="ps_o", bufs=1, space="PSUM"))

        # --- DMA loads (two hwdge queues: SP carries the K side, Act the Q
        # side, so K-proj and Q-proj inputs stream in parallel) ---
        def load2(name, src, width, dt=F32R, dma_eng=None):
            # one DMA for both 128-row tiles: small weight loads are
            # descriptor-bound (max(500ns, transfer)), so merging halves cost
            tl = const.tile([P, 2, width], dt, tag=name, name=name)
            (dma_eng or nc.sync).dma_start(
                out=tl, in_=src.rearrange("(t p) c -> p t c", p=P)
            )
            return [tl[:, t, :] for t in range(2)]

        wk_sb = load2("wk", wkT, HID, dt=BF16)
        warm_in = const.tile([P, 1], F32, tag="warm_in", name="warm_in")
        nc.gpsimd.memset(warm_in, 0.0)
        warm_out = const.tile([P, 1], BF16, tag="warm_out", name="warm_out")
        nc.scalar.activation(warm_out, warm_in, EXP)
        # x tiles hold both 128-row halves ([P, 2, N]) so one DMA chunk
        # carries exactly one matmul's worth of contraction input
        xk2 = big.tile([P, 2, NK], BF16, tag="xk", name="xk")
        xq2 = big.tile([P, 2, NQ], BF16, tag="xq", name="xq")
        xk_sb = [xk2[:, t, :] for t in range(2)]
        xq_sb = [xq2[:, t, :] for t in range(2)]
        xkT_r = xkT.rearrange("(t p) c -> p t c", p=P)
        xqT_r = xqT.rearrange("(t p) c -> p t c", p=P)

        wq_sb = load2("wq", wqT, HID, dt=BF16, dma_eng=nc.scalar)
        for c in range(2):
            nc.scalar.dma_start(
                out=xq2[:, :, c * QBLK : (c + 1) * QBLK],
                in_=xqT_r[:, :, c * QBLK : (c + 1) * QBLK],
            )
        for c in range(NK // QBLK):
            nc.sync.dma_start(
                out=xk2[:, :, c * QBLK : (c + 1) * QBLK],
                in_=xkT_r[:, :, c * QBLK : (c + 1) * QBLK],
            )
        wv_sb = load2("wv", wvT, HID, dt=BF16)

        amf_r = amf.rearrange("(t p) q -> p t q", p=P)
        amq_r = amq.rearrange("(t p) q -> p t q", p=P)
        am_sb = {}   # (qb, pj) -> tile [P, 2, QBLK] bf16
        amq_sb = {}  # (qb, pj) -> tile [P, 2, QBLK] i16

        def load_mask_chunks(qb, dma_eng=None):
            # one chunk per consumption unit, in pair order, so the DMA
            # arrivals pace the pipeline.  qb0's chunks ride the gpsimd
            # queue (idle during the prologue) so the SP queue stays short.
            # qb1's chunks prefetch into their own pool while SP is idle.
            dma_eng = dma_eng or nc.sync
            pool = amp
            qsl = slice(qb * QBLK, (qb + 1) * QBLK)
            for pj in range(NPAIR):
                eng = PAIR_ENG_QB[qb][pj]
                if eng == "D":
                    tq = pool.tile(
                        [P, 2, QBLK], I16, tag=f"amq_{pj}", name=f"amq_{pj}"
                    )
                    dma_eng.dma_start(out=tq, in_=amq_r[:, 2 * pj : 2 * pj + 2, qsl])
                    amq_sb[(qb, pj)] = tq
                else:
                    tl = pool.tile(
                        [P, 2, QBLK], BF16, tag=f"am_{pj}", name=f"am_{pj}"
                    )
                    dma_eng.dma_start(out=tl, in_=amf_r[:, 2 * pj : 2 * pj + 2, qsl])
                    am_sb[(qb, pj)] = tl

        load_mask_chunks(0, dma_eng=nc.gpsimd)
        wo_sb = load2("wo", woT, HID, dt=BF16)

        ident_f = const.tile([P, P], F32, tag="ident_f", name="ident_f")
        make_identity(nc, ident_f)
        ident = const.tile([P, P], BF16, tag="ident", name="ident")
        nc.vector.tensor_copy(ident, ident_f)

        # --- projections (copies alternate Act/DVE; prologue work) ---
        cp_i = 0

        def copy_eng(out, in_):
            nonlocal cp_i
            cp_i += 1
            if cp_i % 2 == 0:
                nc.scalar.copy(out, in_)
            else:
                nc.vector.tensor_copy(out, in_)

        kt_sb = [big.tile([P, NK], F32R, tag=f"kt{t}", name=f"kt{t}") for t in range(2)]
        for nb in range(NK // (2 * QBLK)):
            for t in range(2):
                ps = ps_st.tile([P, 2 * QBLK], F32, tag="st", name="st")
                for half in range(2):
                    for ct in range(NCT):
                        nc.tensor.matmul(
                            ps[:, half * QBLK : (half + 1) * QBLK],
                            lhsT=wk_sb[ct][:, t * P : (t + 1) * P],
                            rhs=xk_sb[ct][
                                :, (2 * nb + half) * QBLK : (2 * nb + half + 1) * QBLK
                            ],
                            start=(ct == 0),
                            stop=(ct == NCT - 1),
                        )
                copy_eng(kt_sb[t][:, 2 * nb * QBLK : (2 * nb + 2) * QBLK], ps)

        qt_sb = [big.tile([P, NQ], F32R, tag=f"qt{t}", name=f"qt{t}") for t in range(2)]
        for t in range(2):
            ps = ps_st.tile([P, 2 * QBLK], F32, tag="st", name="st")
            for half in range(2):
                for ct in range(NCT):
                    nc.tensor.matmul(
                        ps[:, half * QBLK : (half + 1) * QBLK],
                        lhsT=wq_sb[ct][:, t * P : (t + 1) * P],
                        rhs=xq_sb[ct][:, half * QBLK : (half + 1) * QBLK],
                        start=(ct == 0),
                        stop=(ct == NCT - 1),
                    )
            copy_eng(qt_sb[t], ps)

        # V''[pair][ktok, half, h, 0:64] = V rows (bf16); [..., 64] = 1.0
        vpp2 = [None] * (NKT // 2)
        for kt2 in range(NKT // 2):
            ps = ps_st.tile([P, 2 * QBLK], F32, tag="st", name="st")
            for half in range(2):
                kt = 2 * kt2 + half
                for ct in range(NCT):
                    nc.tensor.matmul(
                        ps[:, half * QBLK : half * QBLK + HID],
                        lhsT=xk_sb[ct][:, kt * P : (kt + 1) * P],
                        rhs=wv_sb[ct],
                        start=(ct == 0),
                        stop=(ct == NCT - 1),
                    )
            tl = big.tile(
                [P, 2, NHEAD, DHEAD + 1], BF16, tag=f"v{kt2}", name=f"v{kt2}"
            )
            copy_eng(
                tl[:, :, :, 0:DHEAD],
                ps.rearrange("p (two q) -> p two q", two=2)[:, :, 0:HID].rearrange(
                    "p two (h d) -> p two h d", h=NHEAD
                ),
            )
            nc.gpsimd.memset(tl[:, :, :, DHEAD : DHEAD + 1], 1.0)
            vpp2[kt2] = tl

        # --- attention ---
        def emit_head(qb, h, pts):
            qsl = slice(qb * QBLK, (qb + 1) * QBLK)
            t, po = h // 2, (h % 2) * DHEAD
            for pj in range(NPAIR):
                ps = ps_st.tile([P, 2 * QBLK], F32, tag="st", name="st")
                for half in range(2):
                    kt = 2 * pj + half
                    nc.tensor.matmul(
                        ps[:, half * QBLK : (half + 1) * QBLK],
                        lhsT=kt_sb[t][po : po + DHEAD, kt * P : (kt + 1) * P],
                        rhs=qt_sb[t][po : po + DHEAD, qsl],
                        start=True,
                        stop=True,
                    )
                eng = PAIR_ENG_QB[qb][pj]
                tg = f"pt{h}{qb % 2}_{pj}"
                if eng == "A":
                    pt = ptp.tile([P, 2 * QBLK], BF16, tag=tg, name=tg)
                    nc.scalar.activation(pt, ps, EXP, scale=ACT_SCALE)
                    nc.gpsimd.tensor_mul(
                        pt.rearrange("p (two q) -> p two q", two=2),
                        pt.rearrange("p (two q) -> p two q", two=2),
                        am_sb[(qb, pj)],
                    )
                    for half in range(2):
                        pts[(h, 2 * pj + half)] = pt[:, half * QBLK : (half + 1) * QBLK]
                else:
                    ie = ptp.tile([P, 2 * QBLK], I16, tag=tg, name=tg)
                    nc.vector.tensor_add(
                        ie.rearrange("p (two q) -> p two q", two=2),
                        ps.rearrange("p (two q) -> p two q", two=2),
                        amq_sb[(qb, pj)],
                    )
                    pv = ie.bitcast(BF16)
                    for half in range(2):
                        pts[(h, 2 * pj + half)] = pv[:, half * QBLK : (half + 1) * QBLK]

        deferred = []

        def run_deferred():
            if deferred:
                deferred.pop(0)()

        def process_qb(qb):
            last = qb == NQB - 1
            if qb > 0:
                load_mask_chunks(qb)
            pts = {}
            ps_avs = {}
            ans = {}

            def get_ps_av(qt):
                if qt not in ps_avs:
                    # last qb: qt1 borrows a freed score slot so its chains
                    # don't serialize behind qt0's norm (shared "a0" tag)
                    if qt < 2 and not (last and qt == 1):
                        ps_avs[qt] = ps_a.tile(
                            [P, NHEAD * (DHEAD + 1)], F32, tag="a0",
                            name="a0", padded_shape=[P, QBLK],
                        )
                    else:
                        # tail-only: borrow an idle score-pool slot
                        big_t = ps_st.tile([P, 2 * QBLK], F32, tag="st", name="st_a")
                        ps_avs[qt] = big_t[:, 0 : NHEAD * (DHEAD + 1)]
                return ps_avs[qt]

            def emit_chain(qt, h):
                ps_av = get_ps_av(qt)
                for kt in range(NKT):
                    nc.tensor.matmul(
                        ps_av[:, h * 65 : (h + 1) * 65],
                        lhsT=pts[(h, kt)][:, qt * P : (qt + 1) * P],
                        rhs=vpp2[kt // 2][:, kt % 2, h, :],
                        start=(kt == 0),
                        stop=(kt == NKT - 1),
                    )

            def emit_norm(qt, on_act=False):
                # on_act: route the per-head normalize multiplies to the Act
                # engine (idle during the last qb's tail) so they overlap
                # DVE's att/ob copies
                ps_av = ps_avs.pop(qt)
                rec = wrk.tile([P, NHEAD], F32, tag=f"rec{qt % 2}", name=f"rec{qt % 2}")
                nc.vector.reciprocal(rec, ps_av[:, DHEAD : NHEAD * 65 : 65])
                an = wrk1.tile([P, HID], BF16, tag=f"an{qt % 2}", name=f"an{qt % 2}")
                for h in range(NHEAD):
                    if on_act:
                        nc.scalar.activation(
                            an[:, h * DHEAD : (h + 1) * DHEAD],
                            ps_av[:, h * 65 : h * 65 + DHEAD],
                            mybir.ActivationFunctionType.Copy,
                            scale=rec[:, h : h + 1],
                        )
                    else:
                        nc.vector.tensor_scalar_mul(
                            an[:, h * DHEAD : (h + 1) * DHEAD],
                            ps_av[:, h * 65 : h * 65 + DHEAD],
                            rec[:, h : h + 1],
                        )
                return an

            def emit_o(qt, an, on_act=False, borrow_ps=False, ob_act=False):
                # borrow_ps: use an idle score-pool slot instead of ps_o so two
                # O-chains can run concurrently (tail only, scores finished).
                # on_act/ob_act: route the att/ob copies to Act to parallelize
                # with DVE's stream.
                if borrow_ps:
                    big_t = ps_st.tile([P, 2 * QBLK], F32, tag="st", name="st_o")
                    o_ps = big_t[:, 0:QBLK]
                else:
                    o_ps = ps_o.tile([P, QBLK], F32, tag="o", name="o_ps")
                for ct in range(NCT):
                    tp = o_ps[:, HID + ct * DHEAD : HID + (ct + 1) * DHEAD].bitcast(BF16)
                    nc.tensor.transpose(tp, an[:, ct * P : (ct + 1) * P], ident)
                att = wrk.tile([P, HID], BF16, tag="att", name="att")
                if on_act:
                    nc.scalar.copy(att, o_ps[:, HID : HID + P].bitcast(BF16))
                else:
                    nc.vector.tensor_copy(att, o_ps[:, HID : HID + P].bitcast(BF16))
                for ct in range(NCT):
                    nc.tensor.matmul(
                        o_ps[:, 0:HID],
                        lhsT=att[:, ct * P : (ct + 1) * P],
                        rhs=wo_sb[ct],
                        start=(ct == 0),
                        stop=(ct == NCT - 1),
                    )
                ob = wrk1.tile([P, HID], F32, tag=f"ob{qt % 2}", name=f"ob{qt % 2}")
                q0 = qb * QBLK + qt * P
                if ob_act == "split":
                    # final output: copy + DMA in halves on both engines /
                    # both hwdge queues so the end-of-kernel DMA drain overlaps
                    nc.scalar.copy(ob[:, 0 : HID // 2], o_ps[:, 0 : HID // 2])
                    nc.scalar.dma_start(
                        out=out_d[q0 : q0 + P, 0 : HID // 2], in_=ob[:, 0 : HID // 2]
                    )
                    nc.vector.tensor_copy(
                        ob[:, HID // 2 : HID], o_ps[:, HID // 2 : HID]
                    )
                    nc.sync.dma_start(
                        out=out_d[q0 : q0 + P, HID // 2 : HID],
                        in_=ob[:, HID // 2 : HID],
                    )
                    return
                if ob_act:
                    nc.scalar.copy(ob, o_ps[:, 0:HID])
                else:
                    nc.vector.tensor_copy(ob, o_ps[:, 0:HID])
                dma_eng = nc.scalar if ob_act else nc.sync
                dma_eng.dma_start(out=out_d[q0 : q0 + P, :], in_=ob)

            for h in range(NHEAD):
                emit_head(qb, h, pts)
                if h in (1, 2, 3):
                    # previous qb's deferred tail chains/outputs overlap
                    # this qb's head phases (three slices to spread the
                    # extra PE load; pt tiles are double-buffered across qbs)
                    run_deferred()
                if h > 0:
                    emit_chain(0, h - 1)
            emit_chain(0, NHEAD - 1)
            an0 = emit_norm(0)

            def tail1(emit_chain=emit_chain, emit_norm=emit_norm,
                      emit_o=emit_o, an0=an0, last=last):
                emit_o(0, an0)
                if last:
                    return  # the rest is emitted in tail2's interleaved form
                for h in range(NHEAD):
                    emit_chain(1, h)

            def tail1b(emit_chain=emit_chain, emit_norm=emit_norm,
                       emit_o=emit_o):
                an1 = emit_norm(1)
                emit_o(1, an1)
                for h in range(NHEAD):
                    emit_chain(2, h)

            def tail2(emit_chain=emit_chain, emit_norm=emit_norm,
                      emit_o=emit_o, last=last):
                if not last:
                    an2 = emit_norm(2)
                    for h in range(NHEAD):
                        emit_chain(3, h)
                    emit_o(2, an2)
                    an3 = emit_norm(3)
                    emit_o(3, an3)
                    return
                # Last qb: one interleaved sequence.  Engines are in-order, so
                # emission order per engine is chosen to avoid ready-op
                # head-of-line blocking: DVE gets recips early; norms/copies
                # alternate Act and DVE per qt; PE O-work slots between chain
                # groups.
                for h in range(NHEAD):
                    emit_chain(1, h)
                an1 = emit_norm(1, on_act=True)      # recip1 DVE, norms Act
                for h in range(NHEAD):
                    emit_chain(2, h)
                emit_o(1, an1, on_act=True, ob_act=True)
                an2 = emit_norm(2)                    # recip2+norms DVE
                for h in range(NHEAD):
                    emit_chain(3, h)
                emit_o(2, an2, on_act=True, ob_act=True)  # Act lane
                an3 = emit_norm(3)                    # recip3+norms DVE
                emit_o(3, an3, borrow_ps=True)        # DVE lane, DMA SP

            deferred.append(tail1)
            if not last:
                deferred.append(tail1b)
            deferred.append(tail2)

        for qb in range(NQB):
            process_qb(qb)
        run_deferred()
        run_deferred()
    nc.compile()
    return nc


_NC_CACHE = {}
_last_in_maps = None


def _get_nc(with_attn_mask: bool = False) -> bass.Bass:
    key = "v5"
    if key not in _NC_CACHE:
        _NC_CACHE[key] = build()
    return _NC_CACHE[key]


def kernel(q_hidden_states, k_hidden_states, attention_mask, align_mask, Wq, Wk, Wv, Wo):
    from concourse.bass_utils import run_bass_kernel_spmd

    q_hidden_states = np.asarray(q_hidden_states, np.float32)
    k_hidden_states = np.asarray(k_hidden_states, np.float32)
    attention_mask = np.asarray(attention_mask, np.float32)
    align_mask = np.asarray(align_mask)
    B, Q, _ = q_hidden_states.shape
    qh_len = Q // 2  # 1024

    nc = _get_nc()

    # scores arrive in PSUM pre-scaled by log2e*128 (folded into Wq here)
    wq = np.ascontiguousarray(
        (np.asarray(Wq, np.float32).T * np.float32(FE_MUL / 8.0)).astype(ml_dtypes.bfloat16)
    )
    wk = np.ascontiguousarray(np.asarray(Wk, np.float32).T.astype(ml_dtypes.bfloat16))
    wv = np.ascontiguousarray(np.asarray(Wv, np.float32).T.astype(ml_dtypes.bfloat16))
    wo = np.ascontiguousarray(np.asarray(Wo, np.float32).T.astype(ml_dtypes.bfloat16))

    if np.any(attention_mask):
        raise NotImplementedError("nonzero additive attention_mask not supported")

    in_maps = []
    for core in range(8):
        b, qh = divmod(core, 2)
        qsl = slice(qh * qh_len, (qh + 1) * qh_len)
        am = align_mask[b, :, qsl]
        m = {
            "xqT": np.ascontiguousarray(q_hidden_states[b, qsl].T.astype(ml_dtypes.bfloat16)),
            "xkT": np.ascontiguousarray(k_hidden_states[b].T.astype(ml_dtypes.bfloat16)),
            "amf": np.ascontiguousarray(am.astype(ml_dtypes.bfloat16)),
            "amq": np.ascontiguousarray(
                np.where(am != 0, AMQ_KEEP, AMQ_KILL).astype(np.int16)
            ),
            "wqT": wq,
            "wkT": wk,
            "wvT": wv,
            "woT": wo,
        }
        in_maps.append(m)

    global _last_in_maps
    _last_in_maps = in_maps
    res = run_bass_kernel_spmd(nc, in_maps, list(range(8))).results
    out = np.empty((B, Q, HID), np.float32)
    for core in range(8):
        b, qh = divmod(core, 2)
        out[b, qh * qh_len : (qh + 1) * qh_len] = res[core]["out"]
    return out

